# revision 8
# baseline (speedup 1.0000x reference)
"""Trainium2 Bass kernel for the EpistemicCuriosity module (embedding_lookup).

Data-parallel across 8 NeuronCores: the batch (65536) is split into 8 shards
of 8192 rows; the small MLP weights are replicated. Each core computes

    hidden  = relu(state @ W1_state + W1_act[action] + b1)      # [b, 256]
    pred    = hidden @ W2 + b2                                  # [b, 512]
    pe      = mean((pred - next_state)**2, axis=-1)             # [b]

for its shard, then the per-core sum of pe is AllReduced so every core can
form the updated novelty-buffer mean/std and emit

    nr      = (pe - mean_novelty) / std_novelty

on device. The novelty-buffer scalars (sum, sum-of-squares, replaced value)
are computed host-side from the replicated `novelty_history` input and passed
down as a tiny aux tensor; only pe.mean() needs cross-core communication.

Layout notes (per 512-row group, 4 subtiles of 128 rows):
 - state rows land one-per-partition; features are moved to partitions with
   4 PE transposes per subtile (PSUM), copied back to SBUF by the ACT engine.
 - matmul1 computes hiddenT (hidden units on partitions) so that
   * the W1_act[action] gather (batch-major) is folded in as a PE
     transpose accumulated into the same PSUM group,
   * b1 becomes a per-partition bias applied by the DVE relu,
   * matmul2 needs no further transposes (hiddenT is its lhsT).
 - b2 enters PSUM via a K=1 outer-product matmul with a ones row.
 - (pred - next) on DVE; square+row-sum on ACT via activation(Square,
   scale=1/sqrt(512), accum_out), giving pe directly.
"""

import sys

sys.path.insert(0, "/opt/trn_rl_repo")

from contextlib import ExitStack

import numpy as np

import concourse.bass as bass  # noqa: F401  (registers AP machinery)
import concourse.mybir as mybir
import concourse.tile as tile
from concourse import bacc
from concourse.bass import IndirectOffsetOnAxis
from concourse.bass_utils import run_bass_kernel_spmd
from concourse.masks import make_identity

P = 128
F = 512          # feature dim
H = 256          # hidden dim
V = 5000         # vocab size
HIST = 1000      # novelty history length
N_CORES = 8
B = 65536
B_LOC = B // N_CORES

_BUILD_CACHE = {}


def build_nc(b_loc=B_LOC):
    if b_loc in _BUILD_CACHE:
        return _BUILD_CACHE[b_loc]

    assert b_loc % 512 == 0
    n_groups = b_loc // 512          # 512 rows per DMA group
    ncols = b_loc // P               # pe columns (one per 128-row subtile)

    nc = bacc.Bacc("TRN2", target_bir_lowering=False, debug=False,
                   num_devices=N_CORES)
    f32 = mybir.dt.float32
    i32 = mybir.dt.int32
    Alu = mybir.AluOpType
    Act = mybir.ActivationFunctionType

    state = nc.dram_tensor("state", [b_loc, F], f32, kind="ExternalInput")
    nxt = nc.dram_tensor("next_state", [b_loc, F], f32, kind="ExternalInput")
    action = nc.dram_tensor("action", [b_loc], i32, kind="ExternalInput")
    w1s = nc.dram_tensor("w1_state", [F, H], f32, kind="ExternalInput")
    w1a = nc.dram_tensor("w1_act", [V, H], f32, kind="ExternalInput")
    b1 = nc.dram_tensor("b1", [H], f32, kind="ExternalInput")
    w2 = nc.dram_tensor("w2", [H, F], f32, kind="ExternalInput")
    b2 = nc.dram_tensor("b2", [F], f32, kind="ExternalInput")
    # aux = [S, Q - v^2, v, 0...] from the novelty history (host-computed)
    aux = nc.dram_tensor("aux", [8], f32, kind="ExternalInput")
    pe_out = nc.dram_tensor("pe_out", [b_loc], f32, kind="ExternalOutput")
    nr_out = nc.dram_tensor("nr_out", [b_loc], f32, kind="ExternalOutput")

    f32r = mybir.dt.float32r
    with tile.TileContext(nc) as tc, ExitStack() as ctx:
        const = ctx.enter_context(tc.tile_pool(name="const", bufs=1))
        sbuf = ctx.enter_context(tc.tile_pool(name="sbuf", bufs=5))
        sb2 = ctx.enter_context(tc.tile_pool(name="sb2", bufs=2))
        dram = ctx.enter_context(tc.tile_pool(name="dram", bufs=1, space="DRAM"))

        ident = const.tile([P, P], f32)
        make_identity(nc, ident[:])
        # fp32 weights staged, then rounded to f32r (single-pass PE matmuls)
        w1s_st = const.tile([P, 4, H], f32)
        nc.sync.dma_start(out=w1s_st[:], in_=w1s[:].rearrange("(k p) h -> p k h", p=P))
        w1s_r = const.tile([P, 4, H], f32r)
        nc.vector.tensor_copy(out=w1s_r[:], in_=w1s_st[:])
        w2_st = const.tile([P, 2, F], f32)
        nc.sync.dma_start(out=w2_st[:], in_=w2[:].rearrange("(j p) f -> p j f", p=P))
        w2_r = const.tile([P, 2, F], f32r)
        nc.vector.tensor_copy(out=w2_r[:], in_=w2_st[:])
        b1_sb = const.tile([P, 2], f32)
        nc.sync.dma_start(out=b1_sb[:], in_=b1[:].rearrange("(m p) -> p m", p=P))
        b2_st = const.tile([1, F], f32)
        nc.sync.dma_start(out=b2_st[:], in_=b2[:][None, :])
        b2_r = const.tile([1, F], f32r)
        nc.vector.tensor_copy(out=b2_r[:], in_=b2_st[:])
        aux_sb = const.tile([1, 8], f32)
        nc.sync.dma_start(out=aux_sb[:], in_=aux[:][None, :])
        ones_row = const.tile([1, P], f32)
        nc.vector.memset(ones_row[:], 1.0)
        ones_r = const.tile([1, P], f32r)
        nc.vector.tensor_copy(out=ones_r[:], in_=ones_row[:])
        ones_col = const.tile([P, 1], f32)
        nc.vector.memset(ones_col[:], 1.0)
        pe_all = const.tile([P, ncols], f32)

        # Warm up the collectives machinery while compute runs: a dummy
        # 32-byte AllReduce issued up-front so the real one at the tail
        # doesn't pay ncfw first-call latency.
        warm_sb = const.tile([1, 8], f32)
        nc.vector.memset(warm_sb[:], 0.0)
        warm_in = dram.tile([1, 8], f32)
        warm_out = dram.tile([8, 8], f32)
        nc.gpsimd.dma_start(out=warm_in[:], in_=warm_sb[:])
        nc.gpsimd.collective_compute(
            "AllGather", Alu.bypass,
            replica_groups=[list(range(N_CORES))],
            ins=[warm_in[0:1].opt()], outs=[warm_out.opt()])

        state_h = state[:].rearrange("(g c p) f -> g p c f", c=4, p=P)
        next_h = nxt[:].rearrange("(g c p) f -> g p c f", c=4, p=P)
        act_h = action[:].rearrange("(g c p) -> g p c", c=4, p=P)

        psum = ctx.enter_context(tc.tile_pool(name="psum", bufs=1, space="PSUM"))
        psum2 = ctx.enter_context(tc.tile_pool(name="psum2", bufs=2, space="PSUM"))
        if True:
            for g in range(n_groups):
                st_g = sbuf.tile([P, 4, F], f32, tag="st")
                nc.sync.dma_start(out=st_g[:], in_=state_h[g])
                nx_g = sbuf.tile([P, 4, F], f32, tag="nx")
                nc.scalar.dma_start(out=nx_g[:], in_=next_h[g])
                act_g = sbuf.tile([P, 4], i32, tag="act")
                nc.sync.dma_start(out=act_g[:], in_=act_h[g])
                # NOTE: multi-column offset APs mis-gather on HW (only
                # CoreSim accepts them) — one indirect DMA per 128 rows.
                emb_g = sbuf.tile([P, 4, H], f32, tag="emb")
                for c in range(4):
                    nc.gpsimd.indirect_dma_start(
                        out=emb_g[:, c, :], out_offset=None,
                        in_=w1a[:],
                        in_offset=IndirectOffsetOnAxis(ap=act_g[:, c:c + 1],
                                                       axis=0))

                # stT[k] = [128 feat, 512 batch] via 16 PE transposes (f32);
                # ACT copies round PSUM -> SBUF f32r for the matmuls.
                pstk = [psum.tile([P, F], f32, tag=f"stk{k}", name=f"pstk{k}")
                        for k in range(4)]
                for c in range(4):
                    for k in range(4):
                        nc.tensor.transpose(out=pstk[k][:, c * P:(c + 1) * P],
                                            in_=st_g[:, c, k * P:(k + 1) * P],
                                            identity=ident[:])
                stT_r = sb2.tile([P, 4, F], f32r, tag="stT")
                for k in range(4):
                    nc.scalar.copy(out=stT_r[:, k, :], in_=pstk[k][:])

                # hiddenT (pre-relu): one N=512 f32r matmul per (m, k),
                # embedding rows folded in as f32 PE transposes.
                phid = [psum.tile([P, F], f32, tag=f"phid{m}", name=f"phid{m}")
                        for m in range(2)]
                for m in range(2):
                    for k in range(4):
                        nc.tensor.matmul(out=phid[m][:],
                                         lhsT=w1s_r[:, k, m * P:(m + 1) * P],
                                         rhs=stT_r[:, k, :],
                                         start=(k == 0), stop=False)
                    for c in range(4):
                        nc.tensor.matmul(out=phid[m][:, c * P:(c + 1) * P],
                                         lhsT=emb_g[:, c, m * P:(m + 1) * P],
                                         rhs=ident[:], is_transpose=True,
                                         start=False, stop=(c == 3))

                # relu(x + b1) on DVE (b1 is per-partition here), out f32r
                hidT_r = sb2.tile([P, 2, F], f32r, tag="hidT")
                for m in range(2):
                    nc.vector.tensor_scalar(out=hidT_r[:, m, :], in0=phid[m][:],
                                            scalar1=b1_sb[:, m:m + 1],
                                            scalar2=0.0,
                                            op0=Alu.add, op1=Alu.max)

                for c in range(4):
                    # pred = hiddenT.T @ W2 + b2 (b2 via K=1 outer product)
                    p2 = psum2.tile([P, F], f32, tag="p2")
                    nc.tensor.matmul(out=p2[:], lhsT=ones_r[:], rhs=b2_r[:],
                                     start=True, stop=False)
                    for j in range(2):
                        nc.tensor.matmul(out=p2[:],
                                         lhsT=hidT_r[:, j, c * P:(c + 1) * P],
                                         rhs=w2_r[:, j, :],
                                         start=False, stop=(j == 1))

                    # pe = sum(((pred - next)/sqrt(F))^2) along the row
                    terr = sb2.tile([P, F], f32, tag="terr")
                    nc.vector.tensor_tensor(out=terr[:], in0=p2[:],
                                            in1=nx_g[:, c, :], op=Alu.subtract)
                    sq = sb2.tile([P, F], f32, tag="sq")
                    col = g * 4 + c
                    nc.scalar.activation(out=sq[:], in_=terr[:],
                                         func=Act.Square,
                                         scale=float(1.0 / np.sqrt(F)),
                                         accum_out=pe_all[:, col:col + 1])

        # prediction_error shard out (device layout [p, x]; host reorders)
        nc.sync.dma_start(out=pe_out[:].rearrange("(p x) -> p x", p=P),
                          in_=pe_all[:])

        # per-core sum of pe -> AllReduce -> global sum
        rowsum = const.tile([P, 1], f32)
        nc.vector.tensor_reduce(out=rowsum[:], in_=pe_all[:],
                                axis=mybir.AxisListType.X, op=Alu.add)
        pscal = psum.tile([P, 2], f32, tag="stk0", name="pscal")
        nc.tensor.matmul(out=pscal[0:1, 0:1], lhsT=rowsum[:], rhs=ones_col[:],
                         start=True, stop=True)
        cin_sb = const.tile([1, 8], f32)
        nc.vector.memset(cin_sb[:], 0.0)
        nc.vector.tensor_copy(out=cin_sb[:, 0:1], in_=pscal[0:1, 0:1])
        cc_in = dram.tile([1, 8], f32)
        cc_out = dram.tile([8, 8], f32)
        nc.gpsimd.dma_start(out=cc_in[:], in_=cin_sb[:])
        nc.gpsimd.collective_compute(
            "AllGather", Alu.bypass,
            replica_groups=[list(range(N_CORES))],
            ins=[cc_in[0:1].opt()], outs=[cc_out.opt()])
        parts_sb = const.tile([1, N_CORES], f32)
        nc.gpsimd.dma_start(out=parts_sb[:], in_=cc_out[:, 0][None, :])
        gsum = const.tile([1, 1], f32, tag="gsum")
        nc.vector.tensor_reduce(out=gsum[:], in_=parts_sb[:],
                                axis=mybir.AxisListType.X, op=Alu.add)

        # novelty-buffer stats from scalars (everything [1,1] on partition 0):
        #   m      = global_sum / B
        #   S'     = S - v + m            (updated buffer sum)
        #   sumsq' = (Q - v^2) + m^2      (updated buffer sum of squares)
        #   var'   = sumsq' - S'^2/HIST
        #   std    = max(sqrt(var'/(HIST-1)), 1e-4)
        #   nr     = pe * (1/std) + (-S'/HIST/std)
        S_ap = aux_sb[:, 0:1]
        Qv_ap = aux_sb[:, 1:2]
        v_ap = aux_sb[:, 2:3]
        m_t = const.tile([1, 1], f32, tag="m_t")
        nc.vector.tensor_scalar(out=m_t[:], in0=gsum[:],
                                scalar1=float(1.0 / (b_loc * N_CORES)),
                                scalar2=None, op0=Alu.mult)
        sp_t = const.tile([1, 1], f32, tag="sp_t")
        nc.vector.tensor_scalar(out=sp_t[:], in0=m_t[:], scalar1=v_ap,
                                scalar2=S_ap, op0=Alu.subtract, op1=Alu.add)
        m2_t = const.tile([1, 1], f32, tag="m2_t")
        nc.vector.tensor_tensor(out=m2_t[:], in0=m_t[:], in1=m_t[:], op=Alu.mult)
        ss_t = const.tile([1, 1], f32, tag="ss_t")
        nc.vector.tensor_scalar(out=ss_t[:], in0=m2_t[:], scalar1=Qv_ap,
                                scalar2=None, op0=Alu.add)
        sp2_t = const.tile([1, 1], f32, tag="sp2_t")
        nc.vector.tensor_tensor(out=sp2_t[:], in0=sp_t[:], in1=sp_t[:], op=Alu.mult)
        var_t = const.tile([1, 1], f32, tag="var_t")
        nc.vector.tensor_scalar(out=var_t[:], in0=sp2_t[:],
                                scalar1=float(-1.0 / HIST), scalar2=ss_t[:, 0:1],
                                op0=Alu.mult, op1=Alu.add)
        nc.vector.tensor_scalar(out=var_t[:], in0=var_t[:], scalar1=0.0,
                                scalar2=None, op0=Alu.max)
        std_t = const.tile([1, 1], f32, tag="std_t")
        nc.scalar.activation(out=std_t[:], in_=var_t[:], func=Act.Sqrt,
                             scale=float(1.0 / (HIST - 1)))
        nc.vector.tensor_scalar(out=std_t[:], in0=std_t[:], scalar1=1e-4,
                                scalar2=None, op0=Alu.max)
        inv_t = const.tile([1, 1], f32, tag="inv_t")
        nc.vector.reciprocal(out=inv_t[:], in_=std_t[:])
        bias_t = const.tile([1, 1], f32, tag="bias_t")
        nc.vector.tensor_scalar(out=bias_t[:], in0=sp_t[:], scalar1=inv_t[:, 0:1],
                                scalar2=float(-1.0 / HIST),
                                op0=Alu.mult, op1=Alu.mult)
        pair = const.tile([1, 2], f32, tag="pair")
        nc.vector.tensor_copy(out=pair[:, 0:1], in_=inv_t[:])
        nc.vector.tensor_copy(out=pair[:, 1:2], in_=bias_t[:])

        # broadcast (1/std, -mean/std) to all partitions via a K=1 matmul
        pbc = psum.tile([P, 2], f32, tag="stk1", name="pbc")
        nc.tensor.matmul(out=pbc[:], lhsT=ones_row[:], rhs=pair[:],
                         start=True, stop=True)
        bc_sb = const.tile([P, 2], f32)
        nc.vector.tensor_copy(out=bc_sb[:], in_=pbc[:])

        nr_all = const.tile([P, ncols], f32)
        nc.vector.tensor_scalar(out=nr_all[:], in0=pe_all[:],
                                scalar1=bc_sb[:, 0:1], scalar2=bc_sb[:, 1:2],
                                op0=Alu.mult, op1=Alu.add)
        nc.sync.dma_start(out=nr_out[:].rearrange("(p x) -> p x", p=P),
                          in_=nr_all[:])

    nc.compile()
    _BUILD_CACHE[b_loc] = nc
    return nc


def _make_in_maps(state, action, next_state, novelty_history, history_idx,
                  W1_state, W1_act, b1, W2, b2, b_loc=B_LOC):
    state = np.ascontiguousarray(np.asarray(state, dtype=np.float32))
    next_state = np.ascontiguousarray(np.asarray(next_state, dtype=np.float32))
    action = np.ascontiguousarray(np.asarray(action).astype(np.int32))
    w1s = np.ascontiguousarray(np.asarray(W1_state, dtype=np.float32))
    w1a = np.ascontiguousarray(np.asarray(W1_act, dtype=np.float32))
    b1 = np.ascontiguousarray(np.asarray(b1, dtype=np.float32))
    w2 = np.ascontiguousarray(np.asarray(W2, dtype=np.float32))
    b2 = np.ascontiguousarray(np.asarray(b2, dtype=np.float32))
    nh = np.asarray(novelty_history, dtype=np.float32)

    idx = int(np.asarray(history_idx)) % HIST
    v = np.float32(nh[idx])
    S = np.float32(nh.sum(dtype=np.float32))
    Q = np.float32((nh.astype(np.float32) ** 2).sum(dtype=np.float32))
    aux = np.zeros(8, dtype=np.float32)
    aux[0] = S
    aux[1] = Q - v * v
    aux[2] = v

    in_maps = []
    for i in range(N_CORES):
        sl = slice(i * b_loc, (i + 1) * b_loc)
        in_maps.append({
            "state": state[sl],
            "next_state": next_state[sl],
            "action": action[sl],
            "w1_state": w1s,
            "w1_act": w1a,
            "b1": b1,
            "w2": w2,
            "b2": b2,
            "aux": aux,
        })
    return in_maps


def _unshard(results, b_loc=B_LOC):
    ncols = b_loc // P
    pe_parts, nr_parts = [], []
    for r in results:
        # device layout: element [p, x] = row x*128+p of the shard
        pe_parts.append(r["pe_out"].reshape(P, ncols).T.ravel())
        nr_parts.append(r["nr_out"].reshape(P, ncols).T.ravel())
    return (np.ascontiguousarray(np.concatenate(pe_parts)),
            np.ascontiguousarray(np.concatenate(nr_parts)))


def kernel(state, action, next_state, novelty_history, history_idx,
           W1_state, W1_act, b1, W2, b2):
    nc = build_nc(B_LOC)
    in_maps = _make_in_maps(state, action, next_state, novelty_history,
                            history_idx, W1_state, W1_act, b1, W2, b2)
    res = run_bass_kernel_spmd(nc, in_maps, core_ids=list(range(N_CORES)))
    return _unshard(res.results)


def kernel_traced(state, action, next_state, novelty_history, history_idx,
                  W1_state, W1_act, b1, W2, b2, **spmd_kwargs):
    """Like kernel() but returns (outputs, BassKernelResults) for profiling."""
    nc = build_nc(B_LOC)
    in_maps = _make_in_maps(state, action, next_state, novelty_history,
                            history_idx, W1_state, W1_act, b1, W2, b2)
    res = run_bass_kernel_spmd(nc, in_maps, core_ids=list(range(N_CORES)),
                               **spmd_kwargs)
    return _unshard(res.results), res
